# revision 22
# baseline (speedup 1.0000x reference)
"""Channel-attention (CAM) kernel for Trainium2, 8 NeuronCores.

Reference computation (per batch b):
    A   = x[b].reshape(L, C)            # L = 48^3 = 110592, C = 256
    G   = A^T A                          # [C, C] Gram matrix
    S   = softmax(G, axis=-1)
    out = gamma * (A @ S) + x[b]

Sharding: L-parallel across the 8 cores (each core owns L/8 rows of
both batches).  Each core computes a partial Gram over its shard; a
per-batch bf16 AllReduce completes the [C, C] Grams; every core
redundantly computes softmax (tiny) and its shard of the output.

Structure (phase 1 load-bound, phase 2 PE/store-bound):
  * Residual folded into the second matmul: out = A @ (gamma*S + I),
    so the output comes straight out of PSUM (the PE adds bf16(x)
    exactly in f32) and no separate residual pass exists.
  * x is read from HBM exactly once as [128, 12, 256] supertiles
    (1.5 MB transfers), converted once to bf16, transposed on the PE
    (identity matmuls) into resident A^T tiles; phase 2 re-reads
    nothing.  Transposes of the last NDEF supertiles per batch are
    deferred to phase 2 to fill the PE while the first AllReduce
    completes (the ncfw collective path costs ~40us enqueue-to-start
    plus ~20us run, so softmax weights arrive well after staging).
  * Output is stored bf16 (bf16-exact when gamma == 0) and widened to
    f32 on the host during unsharding.
  * Queue discipline: sync = x loads + output stores; scalar = A^T and
    y PSUM drains + softmax exp; vector = bf16 converts + drain share +
    softmax vector ops; gpsimd = Gram staging DMA, both AllReduces and
    the gf readbacks, so no compute queue ever head-of-line blocks on a
    collective.  softmax compute ops are emitted only at points where
    their inputs are already available, with explicit ordering pins.
"""

import numpy as np
from contextlib import ExitStack

import concourse.bass as bass
import concourse.tile as tile
from concourse.bass import _add_dep_helper
from concourse import bacc, mybir
from concourse.bass import ts
from concourse.bass_utils import run_bass_kernel_spmd
from concourse.masks import make_identity

F32 = mybir.dt.float32
BF16 = mybir.dt.bfloat16
AF = mybir.ActivationFunctionType

N_CORES = 8
B = 2
L = 48 * 48 * 48
C = 256
L_SH = L // N_CORES
ROWS = B * L_SH
P = 128
RPP = 12
SROWS = P * RPP
SPB = L_SH // SROWS
S_TOT = B * SPB
NDEF = 3
TGRP = 2
YGRP = 2

DEFER = {b * SPB + si for b in range(B) for si in range(SPB - NDEF, SPB)}

_CACHE: dict = {}


def _build():
    nc = bacc.Bacc(
        "TRN2", target_bir_lowering=False, debug=False, num_devices=N_CORES
    )
    x_dram = nc.dram_tensor("x", [ROWS, C], F32, kind="ExternalInput")
    g_dram = nc.dram_tensor("gamma", [1, 1], F32, kind="ExternalInput")
    o_dram = nc.dram_tensor("out", [ROWS, C], BF16, kind="ExternalOutput")
    cc_in = [
        nc.dram_tensor(f"cc_in{b}", [2 * P, C], BF16, kind="Internal")
        for b in range(B)
    ]
    cc_out = [
        nc.dram_tensor(f"cc_out{b}", [2 * P, C], BF16, kind="Internal")
        for b in range(B)
    ]
    X, GAM, OUT = x_dram.ap(), g_dram.ap(), o_dram.ap()

    def x_super(s):
        return X[ts(s, SROWS), :].rearrange("(p j) c -> p j c", j=RPP)

    def o_super(s):
        return OUT[ts(s, SROWS), :].rearrange("(p j) c -> p j c", j=RPP)

    with tile.TileContext(nc) as tc, ExitStack() as octx:
        constp = octx.enter_context(tc.tile_pool(name="const", bufs=1))
        ident = constp.tile([P, P], BF16, name="ident", tag="ident")
        make_identity(nc, ident[:])
        gam_sb = constp.tile([1, 1], F32, name="gam_sb", tag="gam_sb")
        nc.sync.dma_start(gam_sb[:], GAM[:, :])
        gam_bc = constp.tile([P, 1], F32, name="gam_bc", tag="gam_bc")
        nc.gpsimd.partition_broadcast(gam_bc[:], gam_sb[:])
        s_bf = [
            constp.tile([P, C], BF16, name=f"sbf{i}", tag=f"sbf{i}")
            for i in range(4)
        ]
        gf = [
            constp.tile([P, C], BF16, name=f"gf{i}", tag=f"gf{i}")
            for i in range(4)
        ]

        atp = octx.enter_context(tc.tile_pool(name="at", bufs=S_TOT))
        at: dict = {}
        for s in range(S_TOT):
            at[s] = atp.tile([P, 2 * RPP, P], BF16, name="atr", tag="atr")

        xbres_pool = octx.enter_context(tc.tile_pool(name="xbres", bufs=2 * NDEF))
        xbres: dict = {}

        smst = octx.enter_context(ExitStack())
        sp = smst.enter_context(tc.tile_pool(name="smx", bufs=2))

        pending = {"v": [], "a": [], "p": []}

        def order_after(inst, key, why):
            for dep_inst in pending[key]:
                _add_dep_helper(inst.ins, dep_inst.ins, sync=False, reason=why)
            pending[key] = []

        def softmax_load(b):
            for m in range(2):
                nc.gpsimd.dma_start(
                    gf[2 * b + m][:], cc_out[b].ap()[ts(m, P), :]
                )

        def softmax_compute(b):
            for m in range(2):
                i = 2 * b + m
                nmx = sp.tile([P, 1], F32, name="nmx", tag="nmx")
                nmxi = nc.vector.tensor_reduce(
                    nmx[:],
                    gf[i][:],
                    axis=mybir.AxisListType.X,
                    op=mybir.AluOpType.max,
                    negate=True,
                )
                if m == 0:
                    order_after(nmxi, "v", f"softmax{b} after pre-AR vector work")
                ex = sp.tile([P, C], F32, name="ex", tag="ex")
                ssum = sp.tile([P, 1], F32, name="ssum", tag="ssum")
                exi = nc.scalar.activation(
                    ex[:], gf[i][:], AF.Exp, bias=nmx[:], scale=1.0,
                    accum_out=ssum[:],
                )
                if m == 0:
                    order_after(exi, "a", f"softmax{b} after pre-AR scalar work")
                inv = sp.tile([P, 1], F32, name="inv", tag="inv")
                nc.vector.reciprocal(inv[:], ssum[:])
                sc = sp.tile([P, 1], F32, name="sc", tag="sc")
                nc.vector.tensor_mul(sc[:], inv[:], gam_bc[:])
                stmp = sp.tile([P, C], BF16, name="stmp", tag="stmp")
                nc.scalar.activation(stmp[:], ex[:], AF.Copy, scale=sc[:])
                o = (1 - m) * P
                nc.vector.tensor_copy(s_bf[i][:, o : o + P], stmp[:, o : o + P])
                nc.vector.tensor_add(
                    s_bf[i][:, m * P : (m + 1) * P],
                    stmp[:, m * P : (m + 1) * P],
                    ident[:],
                )

        p1 = octx.enter_context(ExitStack())
        xtp = p1.enter_context(tc.tile_pool(name="p1x", bufs=3))
        xbp = p1.enter_context(tc.tile_pool(name="p1b", bufs=2))
        gsp = p1.enter_context(tc.tile_pool(name="p1g", bufs=4))
        psg = p1.enter_context(tc.tile_pool(name="psg", bufs=1, space="PSUM"))
        pst1 = p1.enter_context(tc.tile_pool(name="pst1", bufs=2, space="PSUM"))
        g_ps = [
            psg.tile([P, C], F32, name=f"gps{i}", tag=f"gps{i}")
            for i in range(4)
        ]

        def twork(s, pstp, xb, dve_share=False):
            for g in range(RPP // TGRP):
                tp = pstp.tile([P, 2 * TGRP, P], F32, name="tp", tag="tp")
                for jj in range(TGRP):
                    j = g * TGRP + jj
                    for h in range(2):
                        t = nc.tensor.matmul(
                            tp[:, 2 * jj + h, :],
                            xb[:, j, ts(h, P)],
                            ident[:],
                            start=(h == 0 and jj == 0),
                            stop=(h == 1 and jj == TGRP - 1),
                        )
                        pending["p"].append(t)
                dst = at[s][:, ts(g, 2 * TGRP), :]
                if dve_share and g % 2 == 1:
                    cp = nc.vector.tensor_copy(dst, tp[:])
                    pending["v"].append(cp)
                else:
                    cp = nc.scalar.activation(dst, tp[:], AF.Copy)
                    pending["a"].append(cp)

        def phase1_tile(s):
            b, si = divmod(s, SPB)
            xt = xtp.tile([P, RPP, C], F32, name="x1", tag="x1")
            nc.sync.dma_start(xt[:], x_super(s))
            if s in DEFER:
                xb = xbres_pool.tile([P, RPP, C], BF16, name="xbr", tag="xbr")
                xbres[s] = xb
            else:
                xb = xbp.tile([P, RPP, C], BF16, name="xb1", tag="xb1")
            cv = nc.vector.tensor_copy(xb[:], xt[:])
            pending["v"].append(cv)
            defer = s in DEFER
            for g in range(RPP // TGRP):
                tp = None
                if not defer:
                    tp = pst1.tile([P, 2 * TGRP, P], F32, name="tp", tag="tp")
                for jj in range(TGRP):
                    j = g * TGRP + jj
                    first = si == 0 and j == 0
                    last = si == SPB - 1 and j == RPP - 1
                    for h in range(2):
                        nc.tensor.matmul(
                            g_ps[2 * b + h][:], xb[:, j, ts(h, P)], xb[:, j, :],
                            start=first, stop=last,
                        )
                        if not defer:
                            t = nc.tensor.matmul(
                                tp[:, 2 * jj + h, :],
                                xb[:, j, ts(h, P)],
                                ident[:],
                                start=(h == 0 and jj == 0),
                                stop=(h == 1 and jj == TGRP - 1),
                            )
                            pending["p"].append(t)
                if not defer:
                    cp = nc.scalar.activation(
                        at[s][:, ts(g, 2 * TGRP), :], tp[:], AF.Copy
                    )
                    pending["a"].append(cp)

        def stage_and_ar(b):
            for m in range(2):
                gsb = gsp.tile([P, C], BF16, name="gsb", tag="gsb")
                nc.vector.tensor_copy(gsb[:], g_ps[2 * b + m][:])
                nc.gpsimd.dma_start(cc_in[b].ap()[ts(m, P), :], gsb[:])
            nc.gpsimd.collective_compute(
                "AllReduce",
                mybir.AluOpType.add,
                replica_groups=[list(range(N_CORES))],
                ins=[cc_in[b].ap()[:, :]],
                outs=[cc_out[b].ap()[:, :]],
            )
            softmax_load(b)

        for s in range(SPB):
            phase1_tile(s)
        stage_and_ar(0)
        for s in range(SPB, S_TOT):
            phase1_tile(s)
        stage_and_ar(1)
        softmax_compute(0)
        p1.close()

        with ExitStack() as p2:
            op2 = p2.enter_context(tc.tile_pool(name="p2o", bufs=3))
            psy = p2.enter_context(tc.tile_pool(name="psy", bufs=6, space="PSUM"))
            pst2 = p2.enter_context(tc.tile_pool(name="pst2", bufs=2, space="PSUM"))

            def ywork(s, gate_pe=False):
                b = s // SPB
                ot = op2.tile([P, RPP, C], BF16, name="ot", tag="ot")
                for g in range(RPP // YGRP):
                    y = psy.tile([P, YGRP, C], F32, name="y", tag="y")
                    for jj in range(YGRP):
                        j = g * YGRP + jj
                        ym = nc.tensor.matmul(
                            y[:, jj, :], at[s][:, 2 * j, :], s_bf[2 * b][:],
                            start=True, stop=False,
                        )
                        if gate_pe:
                            order_after(ym, "p", "first Y-MM after pending T")
                            gate_pe = False
                        nc.tensor.matmul(
                            y[:, jj, :], at[s][:, 2 * j + 1, :], s_bf[2 * b + 1][:],
                            start=False, stop=True,
                        )
                    dst = ot[:, ts(g, YGRP), :]
                    if (s + g) % 2 == 0:
                        di = nc.scalar.activation(dst, y[:], AF.Copy)
                        pending["a"].append(di)
                    else:
                        di = nc.vector.tensor_copy(dst, y[:])
                        pending["v"].append(di)
                nc.sync.dma_start(o_super(s), ot[:])

            for s in sorted(d for d in DEFER if d < SPB):
                twork(s, pst2, xbres[s], dve_share=True)
            for s in range(SPB):
                ywork(s, gate_pe=(s == 0))
            for s in sorted(d for d in DEFER if d >= SPB):
                twork(s, pst2, xbres[s], dve_share=True)
            softmax_compute(1)
            for s in range(SPB, S_TOT):
                ywork(s, gate_pe=(s == SPB))
        smst.close()

    nc.compile()
    return nc


def _get_nc():
    if "nc" not in _CACHE:
        _CACHE["nc"] = _build()
    return _CACHE["nc"]


def make_in_maps(inputs):
    x3 = np.asarray(inputs["x"], dtype=np.float32).reshape(B, L, C)
    gam = np.asarray(inputs["gamma"], dtype=np.float32).reshape(1, 1)
    in_maps = []
    for k in range(N_CORES):
        shard = np.ascontiguousarray(
            x3[:, k * L_SH : (k + 1) * L_SH, :]
        ).reshape(ROWS, C)
        in_maps.append({"x": shard, "gamma": gam})
    return in_maps


def kernel(x: np.ndarray, gamma: np.ndarray, **_kw) -> np.ndarray:
    nc = _get_nc()
    orig_shape = np.asarray(x).shape
    in_maps = make_in_maps({"x": x, "gamma": gamma})
    res = run_bass_kernel_spmd(nc, in_maps, core_ids=list(range(N_CORES)))
    out = np.empty((B, L, C), dtype=np.float32)
    for k in range(N_CORES):
        out[:, k * L_SH : (k + 1) * L_SH, :] = (
            res.results[k]["out"].astype(np.float32).reshape(B, L_SH, C)
        )
    return out.reshape(orig_shape)


# revision 24
# speedup vs baseline: 1.0018x; 1.0018x over previous
"""Channel-attention (CAM) kernel for Trainium2, 8 NeuronCores.

Reference computation (per batch b):
    A   = x[b].reshape(L, C)            # L = 48^3 = 110592, C = 256
    G   = A^T A                          # [C, C] Gram matrix
    S   = softmax(G, axis=-1)
    out = gamma * (A @ S) + x[b]

Sharding: L-parallel across the 8 cores (each core owns L/8 rows of
both batches).  Each core computes a partial Gram over its shard; a
per-batch bf16 AllReduce completes the [C, C] Grams; every core
redundantly computes softmax (tiny) and its shard of the output.

Structure (phase 1 load-bound, phase 2 PE/store-bound):
  * Residual folded into the second matmul: out = A @ (gamma*S + I),
    so the output comes straight out of PSUM (the PE adds bf16(x)
    exactly in f32) and no separate residual pass exists.
  * x is read from HBM exactly once as [128, 12, 256] supertiles
    (1.5 MB transfers), converted once to bf16, transposed on the PE
    (identity matmuls) into resident A^T tiles; phase 2 re-reads
    nothing.  Transposes of the last NDEF supertiles per batch are
    deferred to phase 2 to fill the PE while the first AllReduce
    completes (the ncfw collective path costs ~40us enqueue-to-start
    plus ~20us run, so softmax weights arrive well after staging).
  * Output is stored bf16 (bf16-exact when gamma == 0) and widened to
    f32 on the host during unsharding.
  * Queue discipline: sync = x loads + output stores; scalar = A^T and
    y PSUM drains + softmax exp; vector = bf16 converts + drain share +
    softmax vector ops; gpsimd = Gram staging DMA, both AllReduces and
    the gf readbacks, so no compute queue ever head-of-line blocks on a
    collective.  softmax compute ops are emitted only at points where
    their inputs are already available, with explicit ordering pins.
"""

import numpy as np
from contextlib import ExitStack

import concourse.bass as bass
import concourse.tile as tile
from concourse.bass import _add_dep_helper
from concourse import bacc, mybir
from concourse.bass import ts
from concourse.bass_utils import run_bass_kernel_spmd
from concourse.masks import make_identity

F32 = mybir.dt.float32
BF16 = mybir.dt.bfloat16
AF = mybir.ActivationFunctionType

N_CORES = 8
B = 2
L = 48 * 48 * 48
C = 256
L_SH = L // N_CORES
ROWS = B * L_SH
P = 128
RPP = 12
SROWS = P * RPP
SPB = L_SH // SROWS
S_TOT = B * SPB
NDEF = 3
TGRP = 2
YGRP = 2

DEFER = {b * SPB + si for b in range(B) for si in range(SPB - NDEF, SPB)}

_CACHE: dict = {}


def _build():
    nc = bacc.Bacc(
        "TRN2", target_bir_lowering=False, debug=False, num_devices=N_CORES
    )
    x_dram = nc.dram_tensor("x", [ROWS, C], F32, kind="ExternalInput")
    g_dram = nc.dram_tensor("gamma", [1, 1], F32, kind="ExternalInput")
    o_dram = nc.dram_tensor("out", [ROWS, C], BF16, kind="ExternalOutput")
    cc_in = [
        nc.dram_tensor(f"cc_in{b}", [2 * P, C], BF16, kind="Internal")
        for b in range(B)
    ]
    cc_out = [
        nc.dram_tensor(f"cc_out{b}", [2 * P, C], BF16, kind="Internal")
        for b in range(B)
    ]
    X, GAM, OUT = x_dram.ap(), g_dram.ap(), o_dram.ap()

    def x_super(s):
        return X[ts(s, SROWS), :].rearrange("(p j) c -> p j c", j=RPP)

    def o_super(s):
        return OUT[ts(s, SROWS), :].rearrange("(p j) c -> p j c", j=RPP)

    with tile.TileContext(nc) as tc, ExitStack() as octx:
        constp = octx.enter_context(tc.tile_pool(name="const", bufs=1))
        ident = constp.tile([P, P], BF16, name="ident", tag="ident")
        make_identity(nc, ident[:])
        gam_sb = constp.tile([1, 1], F32, name="gam_sb", tag="gam_sb")
        nc.sync.dma_start(gam_sb[:], GAM[:, :])
        gam_bc = constp.tile([P, 1], F32, name="gam_bc", tag="gam_bc")
        nc.gpsimd.partition_broadcast(gam_bc[:], gam_sb[:])
        s_bf = [
            constp.tile([P, C], BF16, name=f"sbf{i}", tag=f"sbf{i}")
            for i in range(4)
        ]
        gf = [
            constp.tile([P, C], BF16, name=f"gf{i}", tag=f"gf{i}")
            for i in range(4)
        ]

        atp = octx.enter_context(tc.tile_pool(name="at", bufs=S_TOT))
        at: dict = {}
        for s in range(S_TOT):
            at[s] = atp.tile([P, 2 * RPP, P], BF16, name="atr", tag="atr")

        xbres_pool = octx.enter_context(tc.tile_pool(name="xbres", bufs=2 * NDEF))
        xbres: dict = {}

        smst = octx.enter_context(ExitStack())
        sp = smst.enter_context(tc.tile_pool(name="smx", bufs=2))

        pending = {"v": [], "a": [], "p": []}

        def order_after(inst, key, why):
            for dep_inst in pending[key]:
                _add_dep_helper(inst.ins, dep_inst.ins, sync=False, reason=why)
            pending[key] = []

        def softmax_load(b):
            for m in range(2):
                nc.gpsimd.dma_start(
                    gf[2 * b + m][:], cc_out[b].ap()[ts(m, P), :]
                )

        def softmax_compute(b):
            for m in range(2):
                i = 2 * b + m
                nmx = sp.tile([P, 1], F32, name="nmx", tag="nmx")
                nmxi = nc.vector.tensor_reduce(
                    nmx[:],
                    gf[i][:],
                    axis=mybir.AxisListType.X,
                    op=mybir.AluOpType.max,
                    negate=True,
                )
                if m == 0:
                    order_after(nmxi, "v", f"softmax{b} after pre-AR vector work")
                ex = sp.tile([P, C], F32, name="ex", tag="ex")
                ssum = sp.tile([P, 1], F32, name="ssum", tag="ssum")
                exi = nc.scalar.activation(
                    ex[:], gf[i][:], AF.Exp, bias=nmx[:], scale=1.0,
                    accum_out=ssum[:],
                )
                if m == 0:
                    order_after(exi, "a", f"softmax{b} after pre-AR scalar work")
                inv = sp.tile([P, 1], F32, name="inv", tag="inv")
                nc.vector.reciprocal(inv[:], ssum[:])
                sc = sp.tile([P, 1], F32, name="sc", tag="sc")
                nc.vector.tensor_mul(sc[:], inv[:], gam_bc[:])
                stmp = sp.tile([P, C], BF16, name="stmp", tag="stmp")
                nc.scalar.activation(stmp[:], ex[:], AF.Copy, scale=sc[:])
                o = (1 - m) * P
                nc.vector.tensor_copy(s_bf[i][:, o : o + P], stmp[:, o : o + P])
                nc.vector.tensor_add(
                    s_bf[i][:, m * P : (m + 1) * P],
                    stmp[:, m * P : (m + 1) * P],
                    ident[:],
                )

        p1 = octx.enter_context(ExitStack())
        xtp = p1.enter_context(tc.tile_pool(name="p1x", bufs=3))
        xbp = p1.enter_context(tc.tile_pool(name="p1b", bufs=2))
        gsp = p1.enter_context(tc.tile_pool(name="p1g", bufs=4))
        psg = p1.enter_context(tc.tile_pool(name="psg", bufs=1, space="PSUM"))
        pst1 = p1.enter_context(tc.tile_pool(name="pst1", bufs=2, space="PSUM"))
        g_ps = [
            psg.tile([P, C], F32, name=f"gps{i}", tag=f"gps{i}")
            for i in range(4)
        ]

        def twork(s, pstp, xb, dve_share=False):
            for g in range(RPP // TGRP):
                tp = pstp.tile([P, 2 * TGRP, P], F32, name="tp", tag="tp")
                for jj in range(TGRP):
                    j = g * TGRP + jj
                    for h in range(2):
                        t = nc.tensor.matmul(
                            tp[:, 2 * jj + h, :],
                            xb[:, j, ts(h, P)],
                            ident[:],
                            start=(h == 0 and jj == 0),
                            stop=(h == 1 and jj == TGRP - 1),
                        )
                        pending["p"].append(t)
                dst = at[s][:, ts(g, 2 * TGRP), :]
                if dve_share and g % 2 == 1:
                    cp = nc.vector.tensor_copy(dst, tp[:])
                    pending["v"].append(cp)
                else:
                    cp = nc.scalar.activation(dst, tp[:], AF.Copy)
                    pending["a"].append(cp)

        def phase1_tile(s):
            b, si = divmod(s, SPB)
            xt = xtp.tile([P, RPP, C], F32, name="x1", tag="x1")
            nc.sync.dma_start(xt[:], x_super(s))
            if s in DEFER:
                xb = xbres_pool.tile([P, RPP, C], BF16, name="xbr", tag="xbr")
                xbres[s] = xb
            else:
                xb = xbp.tile([P, RPP, C], BF16, name="xb1", tag="xb1")
            cv = nc.vector.tensor_copy(xb[:], xt[:])
            pending["v"].append(cv)
            defer = s in DEFER
            for g in range(RPP // TGRP):
                tp = None
                if not defer:
                    tp = pst1.tile([P, 2 * TGRP, P], F32, name="tp", tag="tp")
                for jj in range(TGRP):
                    j = g * TGRP + jj
                    first = si == 0 and j == 0
                    last = si == SPB - 1 and j == RPP - 1
                    for h in range(2):
                        nc.tensor.matmul(
                            g_ps[2 * b + h][:], xb[:, j, ts(h, P)], xb[:, j, :],
                            start=first, stop=last,
                        )
                        if not defer:
                            t = nc.tensor.matmul(
                                tp[:, 2 * jj + h, :],
                                xb[:, j, ts(h, P)],
                                ident[:],
                                start=(h == 0 and jj == 0),
                                stop=(h == 1 and jj == TGRP - 1),
                            )
                            pending["p"].append(t)
                if not defer:
                    cp = nc.scalar.activation(
                        at[s][:, ts(g, 2 * TGRP), :], tp[:], AF.Copy
                    )
                    pending["a"].append(cp)

        def stage_and_ar(b):
            for m in range(2):
                gsb = gsp.tile([P, C], BF16, name="gsb", tag="gsb")
                nc.vector.tensor_copy(gsb[:], g_ps[2 * b + m][:])
                nc.gpsimd.dma_start(cc_in[b].ap()[ts(m, P), :], gsb[:])
            nc.gpsimd.collective_compute(
                "AllReduce",
                mybir.AluOpType.add,
                replica_groups=[list(range(N_CORES))],
                ins=[cc_in[b].ap()[:, :]],
                outs=[cc_out[b].ap()[:, :]],
            )
            softmax_load(b)

        for s in range(SPB):
            phase1_tile(s)
        stage_and_ar(0)
        for s in range(SPB, S_TOT):
            phase1_tile(s)
        stage_and_ar(1)
        p1.close()

        with ExitStack() as p2:
            op2 = p2.enter_context(tc.tile_pool(name="p2o", bufs=3))
            psy = p2.enter_context(tc.tile_pool(name="psy", bufs=6, space="PSUM"))
            pst2 = p2.enter_context(tc.tile_pool(name="pst2", bufs=2, space="PSUM"))

            def ywork(s, gate_pe=False):
                b = s // SPB
                ot = op2.tile([P, RPP, C], BF16, name="ot", tag="ot")
                for g in range(RPP // YGRP):
                    y = psy.tile([P, YGRP, C], F32, name="y", tag="y")
                    for jj in range(YGRP):
                        j = g * YGRP + jj
                        ym = nc.tensor.matmul(
                            y[:, jj, :], at[s][:, 2 * j, :], s_bf[2 * b][:],
                            start=True, stop=False,
                        )
                        if gate_pe:
                            order_after(ym, "p", "first Y-MM after pending T")
                            gate_pe = False
                        nc.tensor.matmul(
                            y[:, jj, :], at[s][:, 2 * j + 1, :], s_bf[2 * b + 1][:],
                            start=False, stop=True,
                        )
                    dst = ot[:, ts(g, YGRP), :]
                    if (s + g) % 2 == 0:
                        di = nc.scalar.activation(dst, y[:], AF.Copy)
                        pending["a"].append(di)
                    else:
                        di = nc.vector.tensor_copy(dst, y[:])
                        pending["v"].append(di)
                nc.sync.dma_start(o_super(s), ot[:])

            # deferred b0 transposes FIRST, then softmax(0): the softmax
            # compute ops wait on the AllReduce, so anything emitted after
            # them on the ACT/DVE queues would head-of-line block until it
            # completes -- keep all AR-independent drains ahead of them
            for s in sorted(d for d in DEFER if d < SPB):
                twork(s, pst2, xbres[s], dve_share=True)
            softmax_compute(0)
            for s in range(SPB):
                ywork(s, gate_pe=(s == 0))
                # softmax(1) mid-b0-leg: the ACT/DVE queues reach these ops
                # just after AllReduce(1) typically lands, so M(1) is ready
                # before the b1 matmuls need it without blocking b0 drains
                if s == SPB - 3:
                    softmax_compute(1)
            for s in sorted(d for d in DEFER if d >= SPB):
                twork(s, pst2, xbres[s], dve_share=True)
            for s in range(SPB, S_TOT):
                ywork(s, gate_pe=(s == SPB))
        smst.close()

    nc.compile()
    return nc


def _get_nc():
    if "nc" not in _CACHE:
        _CACHE["nc"] = _build()
    return _CACHE["nc"]


def make_in_maps(inputs):
    x3 = np.asarray(inputs["x"], dtype=np.float32).reshape(B, L, C)
    gam = np.asarray(inputs["gamma"], dtype=np.float32).reshape(1, 1)
    in_maps = []
    for k in range(N_CORES):
        shard = np.ascontiguousarray(
            x3[:, k * L_SH : (k + 1) * L_SH, :]
        ).reshape(ROWS, C)
        in_maps.append({"x": shard, "gamma": gam})
    return in_maps


def kernel(x: np.ndarray, gamma: np.ndarray, **_kw) -> np.ndarray:
    nc = _get_nc()
    orig_shape = np.asarray(x).shape
    in_maps = make_in_maps({"x": x, "gamma": gamma})
    res = run_bass_kernel_spmd(nc, in_maps, core_ids=list(range(N_CORES)))
    out = np.empty((B, L, C), dtype=np.float32)
    for k in range(N_CORES):
        out[:, k * L_SH : (k + 1) * L_SH, :] = (
            res.results[k]["out"].astype(np.float32).reshape(B, L_SH, C)
        )
    return out.reshape(orig_shape)


# revision 25
# speedup vs baseline: 1.0329x; 1.0310x over previous
"""Channel-attention (CAM) kernel for Trainium2, 8 NeuronCores.

Reference computation (per batch b):
    A   = x[b].reshape(L, C)            # L = 48^3 = 110592, C = 256
    G   = A^T A                          # [C, C] Gram matrix
    S   = softmax(G, axis=-1)
    out = gamma * (A @ S) + x[b]

Sharding: L-parallel across the 8 cores (each core owns L/8 rows of
both batches).  Each core computes a partial Gram over its shard; a
per-batch bf16 AllReduce completes the [C, C] Grams; every core
redundantly computes softmax (tiny) and its shard of the output.

Structure (phase 1 load-bound, phase 2 PE/store-bound):
  * Residual folded into the second matmul: out = A @ (gamma*S + I),
    so the output comes straight out of PSUM (the PE adds bf16(x)
    exactly in f32) and no separate residual pass exists.
  * x is read from HBM exactly once as [128, 12, 256] supertiles
    (1.5 MB transfers), converted once to bf16, transposed on the PE
    (identity matmuls) into resident A^T tiles; phase 2 re-reads
    nothing.  Transposes of the last NDEF supertiles per batch are
    deferred to phase 2 to fill the PE while the first AllReduce
    completes (the ncfw collective path costs ~40us enqueue-to-start
    plus ~20us run, so softmax weights arrive well after staging).
  * Output is stored bf16 (bf16-exact when gamma == 0) and widened to
    f32 on the host during unsharding.
  * Queue discipline: sync = x loads + output stores; scalar = A^T and
    y PSUM drains + softmax exp; vector = bf16 converts + drain share +
    softmax vector ops; gpsimd = Gram staging DMA, both AllReduces and
    the gf readbacks, so no compute queue ever head-of-line blocks on a
    collective.  softmax compute ops are emitted only at points where
    their inputs are already available, with explicit ordering pins.
"""

import numpy as np
from contextlib import ExitStack

import concourse.bass as bass
import concourse.tile as tile
from concourse.bass import _add_dep_helper
from concourse import bacc, mybir
from concourse.bass import ts
from concourse.bass_utils import run_bass_kernel_spmd
from concourse.masks import make_identity

F32 = mybir.dt.float32
BF16 = mybir.dt.bfloat16
AF = mybir.ActivationFunctionType

N_CORES = 8
B = 2
L = 48 * 48 * 48
C = 256
L_SH = L // N_CORES
ROWS = B * L_SH
P = 128
RPP = 12
SROWS = P * RPP
SPB = L_SH // SROWS
S_TOT = B * SPB
NDEF = 3
TGRP = 2
YGRP = 2

DEFER = {b * SPB + si for b in range(B) for si in range(SPB - NDEF, SPB)}

_CACHE: dict = {}


def _build():
    nc = bacc.Bacc(
        "TRN2", target_bir_lowering=False, debug=False, num_devices=N_CORES
    )
    x_dram = nc.dram_tensor("x", [ROWS, C], F32, kind="ExternalInput")
    g_dram = nc.dram_tensor("gamma", [1, 1], F32, kind="ExternalInput")
    o_dram = nc.dram_tensor("out", [ROWS, C], BF16, kind="ExternalOutput")
    cc_in = [
        nc.dram_tensor(f"cc_in{b}", [2 * P, C], BF16, kind="Internal")
        for b in range(B)
    ]
    cc_out = [
        nc.dram_tensor(f"cc_out{b}", [2 * P, C], BF16, kind="Internal")
        for b in range(B)
    ]
    X, GAM, OUT = x_dram.ap(), g_dram.ap(), o_dram.ap()

    def x_super(s):
        return X[ts(s, SROWS), :].rearrange("(p j) c -> p j c", j=RPP)

    def o_super(s):
        return OUT[ts(s, SROWS), :].rearrange("(p j) c -> p j c", j=RPP)

    with tile.TileContext(nc) as tc, ExitStack() as octx:
        constp = octx.enter_context(tc.tile_pool(name="const", bufs=1))
        ident = constp.tile([P, P], BF16, name="ident", tag="ident")
        make_identity(nc, ident[:])
        gam_sb = constp.tile([1, 1], F32, name="gam_sb", tag="gam_sb")
        nc.sync.dma_start(gam_sb[:], GAM[:, :])
        gam_bc = constp.tile([P, 1], F32, name="gam_bc", tag="gam_bc")
        nc.gpsimd.partition_broadcast(gam_bc[:], gam_sb[:])
        s_bf = [
            constp.tile([P, C], BF16, name=f"sbf{i}", tag=f"sbf{i}")
            for i in range(4)
        ]
        gf = [
            constp.tile([P, C], BF16, name=f"gf{i}", tag=f"gf{i}")
            for i in range(4)
        ]

        atp = octx.enter_context(tc.tile_pool(name="at", bufs=S_TOT))
        at: dict = {}
        for s in range(S_TOT):
            at[s] = atp.tile([P, 2 * RPP, P], BF16, name="atr", tag="atr")

        xbres_pool = octx.enter_context(tc.tile_pool(name="xbres", bufs=2 * NDEF))
        xbres: dict = {}

        smst = octx.enter_context(ExitStack())
        sp = smst.enter_context(tc.tile_pool(name="smx", bufs=2))

        pending = {"v": [], "a": [], "p": []}

        def order_after(inst, key, why):
            for dep_inst in pending[key]:
                _add_dep_helper(inst.ins, dep_inst.ins, sync=False, reason=why)
            pending[key] = []

        def softmax_load(b):
            for m in range(2):
                nc.gpsimd.dma_start(
                    gf[2 * b + m][:], cc_out[b].ap()[ts(m, P), :]
                )

        def softmax_compute(b):
            for m in range(2):
                i = 2 * b + m
                nmx = sp.tile([P, 1], F32, name="nmx", tag="nmx")
                nmxi = nc.vector.tensor_reduce(
                    nmx[:],
                    gf[i][:],
                    axis=mybir.AxisListType.X,
                    op=mybir.AluOpType.max,
                    negate=True,
                )
                if m == 0:
                    order_after(nmxi, "v", f"softmax{b} after pre-AR vector work")
                ex = sp.tile([P, C], F32, name="ex", tag="ex")
                ssum = sp.tile([P, 1], F32, name="ssum", tag="ssum")
                exi = nc.scalar.activation(
                    ex[:], gf[i][:], AF.Exp, bias=nmx[:], scale=1.0,
                    accum_out=ssum[:],
                )
                if m == 0:
                    order_after(exi, "a", f"softmax{b} after pre-AR scalar work")
                inv = sp.tile([P, 1], F32, name="inv", tag="inv")
                nc.vector.reciprocal(inv[:], ssum[:])
                sc = sp.tile([P, 1], F32, name="sc", tag="sc")
                nc.vector.tensor_mul(sc[:], inv[:], gam_bc[:])
                stmp = sp.tile([P, C], BF16, name="stmp", tag="stmp")
                nc.scalar.activation(stmp[:], ex[:], AF.Copy, scale=sc[:])
                o = (1 - m) * P
                nc.vector.tensor_copy(s_bf[i][:, o : o + P], stmp[:, o : o + P])
                nc.vector.tensor_add(
                    s_bf[i][:, m * P : (m + 1) * P],
                    stmp[:, m * P : (m + 1) * P],
                    ident[:],
                )

        p1 = octx.enter_context(ExitStack())
        xtp = p1.enter_context(tc.tile_pool(name="p1x", bufs=3))
        xbp = p1.enter_context(tc.tile_pool(name="p1b", bufs=2))
        gsp = p1.enter_context(tc.tile_pool(name="p1g", bufs=4))
        psg = p1.enter_context(tc.tile_pool(name="psg", bufs=1, space="PSUM"))
        pst1 = p1.enter_context(tc.tile_pool(name="pst1", bufs=2, space="PSUM"))
        g_ps = [
            psg.tile([P, C], F32, name=f"gps{i}", tag=f"gps{i}")
            for i in range(4)
        ]

        def twork(s, pstp, xb, dve_share=False):
            for g in range(RPP // TGRP):
                tp = pstp.tile([P, 2 * TGRP, P], F32, name="tp", tag="tp")
                for jj in range(TGRP):
                    j = g * TGRP + jj
                    for h in range(2):
                        t = nc.tensor.matmul(
                            tp[:, 2 * jj + h, :],
                            xb[:, j, ts(h, P)],
                            ident[:],
                            start=(h == 0 and jj == 0),
                            stop=(h == 1 and jj == TGRP - 1),
                        )
                        pending["p"].append(t)
                dst = at[s][:, ts(g, 2 * TGRP), :]
                if dve_share and g % 2 == 1:
                    cp = nc.vector.tensor_copy(dst, tp[:])
                    pending["v"].append(cp)
                else:
                    cp = nc.scalar.activation(dst, tp[:], AF.Copy)
                    pending["a"].append(cp)

        def phase1_tile(s):
            b, si = divmod(s, SPB)
            xt = xtp.tile([P, RPP, C], F32, name="x1", tag="x1")
            nc.sync.dma_start(xt[:], x_super(s))
            if s in DEFER:
                xb = xbres_pool.tile([P, RPP, C], BF16, name="xbr", tag="xbr")
                xbres[s] = xb
            else:
                xb = xbp.tile([P, RPP, C], BF16, name="xb1", tag="xb1")
            cv = nc.vector.tensor_copy(xb[:], xt[:])
            pending["v"].append(cv)
            defer = s in DEFER
            for g in range(RPP // TGRP):
                tp = None
                if not defer:
                    tp = pst1.tile([P, 2 * TGRP, P], F32, name="tp", tag="tp")
                for jj in range(TGRP):
                    j = g * TGRP + jj
                    first = si == 0 and j == 0
                    last = si == SPB - 1 and j == RPP - 1
                    for h in range(2):
                        nc.tensor.matmul(
                            g_ps[2 * b + h][:], xb[:, j, ts(h, P)], xb[:, j, :],
                            start=first, stop=last,
                        )
                        if not defer:
                            t = nc.tensor.matmul(
                                tp[:, 2 * jj + h, :],
                                xb[:, j, ts(h, P)],
                                ident[:],
                                start=(h == 0 and jj == 0),
                                stop=(h == 1 and jj == TGRP - 1),
                            )
                            pending["p"].append(t)
                if not defer:
                    cp = nc.scalar.activation(
                        at[s][:, ts(g, 2 * TGRP), :], tp[:], AF.Copy
                    )
                    pending["a"].append(cp)

        def stage_and_ar(b):
            for m in range(2):
                gsb = gsp.tile([P, C], BF16, name="gsb", tag="gsb")
                nc.vector.tensor_copy(gsb[:], g_ps[2 * b + m][:])
                nc.gpsimd.dma_start(cc_in[b].ap()[ts(m, P), :], gsb[:])
            nc.gpsimd.collective_compute(
                "AllReduce",
                mybir.AluOpType.add,
                replica_groups=[list(range(N_CORES))],
                ins=[cc_in[b].ap()[:, :]],
                outs=[cc_out[b].ap()[:, :]],
            )
            softmax_load(b)

        for s in range(SPB):
            phase1_tile(s)
        stage_and_ar(0)
        for s in range(SPB, S_TOT):
            phase1_tile(s)
        stage_and_ar(1)
        p1.close()

        with ExitStack() as p2:
            op2 = p2.enter_context(tc.tile_pool(name="p2o", bufs=3))
            psy = p2.enter_context(tc.tile_pool(name="psy", bufs=6, space="PSUM"))
            pst2 = p2.enter_context(tc.tile_pool(name="pst2", bufs=2, space="PSUM"))

            def ywork(s, gate_pe=False):
                b = s // SPB
                ot = op2.tile([P, RPP, C], BF16, name="ot", tag="ot")
                for g in range(RPP // YGRP):
                    y = psy.tile([P, YGRP, C], F32, name="y", tag="y")
                    for jj in range(YGRP):
                        j = g * YGRP + jj
                        ym = nc.tensor.matmul(
                            y[:, jj, :], at[s][:, 2 * j, :], s_bf[2 * b][:],
                            start=True, stop=False,
                        )
                        if gate_pe:
                            order_after(ym, "p", "first Y-MM after pending T")
                            gate_pe = False
                        nc.tensor.matmul(
                            y[:, jj, :], at[s][:, 2 * j + 1, :], s_bf[2 * b + 1][:],
                            start=False, stop=True,
                        )
                    dst = ot[:, ts(g, YGRP), :]
                    if (s + g) % 2 == 0:
                        di = nc.scalar.activation(dst, y[:], AF.Copy)
                        pending["a"].append(di)
                    else:
                        di = nc.vector.tensor_copy(dst, y[:])
                        pending["v"].append(di)
                    # store each half-supertile as soon as its drains land
                    # so the final store is half-sized and issue overlaps
                    # the later drains
                    if g == RPP // YGRP // 2 - 1:
                        nc.sync.dma_start(
                            o_super(s)[:, 0 : RPP // 2, :],
                            ot[:, 0 : RPP // 2, :],
                        )
                nc.sync.dma_start(
                    o_super(s)[:, RPP // 2 : RPP, :],
                    ot[:, RPP // 2 : RPP, :],
                )

            # deferred b0 transposes FIRST, then softmax(0): the softmax
            # compute ops wait on the AllReduce, so anything emitted after
            # them on the ACT/DVE queues would head-of-line block until it
            # completes -- keep all AR-independent drains ahead of them
            for s in sorted(d for d in DEFER if d < SPB):
                twork(s, pst2, xbres[s], dve_share=True)
            softmax_compute(0)
            for s in range(SPB):
                ywork(s, gate_pe=(s == 0))
                # softmax(1) mid-b0-leg: the ACT/DVE queues reach these ops
                # just after AllReduce(1) typically lands, so M(1) is ready
                # before the b1 matmuls need it without blocking b0 drains
                if s == SPB - 3:
                    softmax_compute(1)
            for s in sorted(d for d in DEFER if d >= SPB):
                twork(s, pst2, xbres[s], dve_share=True)
            for s in range(SPB, S_TOT):
                ywork(s, gate_pe=(s == SPB))
        smst.close()

    nc.compile()
    return nc


def _get_nc():
    if "nc" not in _CACHE:
        _CACHE["nc"] = _build()
    return _CACHE["nc"]


def make_in_maps(inputs):
    x3 = np.asarray(inputs["x"], dtype=np.float32).reshape(B, L, C)
    gam = np.asarray(inputs["gamma"], dtype=np.float32).reshape(1, 1)
    in_maps = []
    for k in range(N_CORES):
        shard = np.ascontiguousarray(
            x3[:, k * L_SH : (k + 1) * L_SH, :]
        ).reshape(ROWS, C)
        in_maps.append({"x": shard, "gamma": gam})
    return in_maps


def kernel(x: np.ndarray, gamma: np.ndarray, **_kw) -> np.ndarray:
    nc = _get_nc()
    orig_shape = np.asarray(x).shape
    in_maps = make_in_maps({"x": x, "gamma": gamma})
    res = run_bass_kernel_spmd(nc, in_maps, core_ids=list(range(N_CORES)))
    out = np.empty((B, L, C), dtype=np.float32)
    for k in range(N_CORES):
        out[:, k * L_SH : (k + 1) * L_SH, :] = (
            res.results[k]["out"].astype(np.float32).reshape(B, L_SH, C)
        )
    return out.reshape(orig_shape)
